# revision 9
# baseline (speedup 1.0000x reference)
"""ArcFace loss kernel for 8 TRN2 NeuronCores.

Strategy: tensor-parallel over classes (C=50000 -> 6250/core, padded to
6656 = 13*512).  Each core computes cos(emb, w_shard) with a bf16 matmul,
a fused exp+row-sum epilogue on the scalar engine, and the label logit via
host-gathered label weight rows (pure indexing) dotted on-device.  A single
8 KB AllReduce combines the per-core sum-exp vectors; the margin-corrected
log-softmax NLL mean is computed redundantly on every core.
"""

import numpy as np

from concourse import bacc, bass, mybir, tile
from concourse import bass_utils
from concourse.bass_interp import get_hw_module

B, D, C = 2048, 512, 50000
NCORES = 8
CS = C // NCORES            # 6250 classes per core
CSP = 6656                  # padded to 13*512
PAD = CSP - CS              # 406 zero-padded classes per core
MARGIN = 0.3
SCALE = 30.0
EPS = 1e-12

F32 = mybir.dt.float32
BF16 = mybir.dt.bfloat16
Act = mybir.ActivationFunctionType
Alu = mybir.AluOpType

NB = B // 128               # 16 batch tiles
NK = D // 128               # 4 contraction tiles
# main-loop column groups (pairs of 512 -> 1024-wide psum tiles)
JGROUPS = [(o, min(1024, CSP - o)) for o in range(0, CSP, 1024)]  # 6x1024 + 1x512
NJ = len(JGROUPS)           # 7
# weight-prep slabs
SLABS = [(o, min(2048, CSP - o)) for o in range(0, CSP, 2048)]    # 3x2048 + 1x512


def build(stage="full"):
    # stage: bisect knob — "prep" (label+weight prep only), "main" (+matmul/exp),
    # "nocc" (full minus collective), "full"
    nc = bacc.Bacc("TRN2", debug=False, num_devices=NCORES)

    embT_d = nc.dram_tensor("embT", [D, B], F32, kind="ExternalInput")
    emb_d = nc.dram_tensor("emb", [B, D], F32, kind="ExternalInput")
    wlab_d = nc.dram_tensor("wlab", [B, D], F32, kind="ExternalInput")
    wT_d = nc.dram_tensor("wT", [D, CSP], F32, kind="ExternalInput")
    out_d = nc.dram_tensor("out", [1, 1], F32, kind="ExternalOutput")

    with tile.TileContext(nc) as tc:
        with (
            tc.tile_pool(name="const", bufs=1) as constp,
            tc.tile_pool(name="res", bufs=1) as resp,
            tc.tile_pool(name="psum_cos", bufs=2, space="PSUM") as pcosp,
            tc.tile_pool(name="dram", bufs=1, space="DRAM") as dramp,
        ):
            ones_st = constp.tile([128, 128], BF16, tag="ones_st")
            nc.vector.memset(ones_st[:], 1.0)
            ones_col = constp.tile([128, 1], F32, tag="ones_col")
            nc.vector.memset(ones_col[:], 1.0)

            # resident tensors
            ebT_bf = resp.tile([128, NK, B], BF16, tag="ebT_bf")      # 16 KB/part
            wtn_bf = resp.tile([128, NK, CSP], BF16, tag="wtn_bf")    # 53 KB/part
            Pcols = resp.tile([128, NB * NJ], F32, tag="Pcols")       # exp-sum accums
            sse_c = resp.tile([128, NB], F32, tag="sse_c")            # ||e||^2
            ssw_c = resp.tile([128, NB], F32, tag="ssw_c")            # ||w_lab||^2
            dot_c = resp.tile([128, NB], F32, tag="dot_c")            # e . w_lab
            cosl_c = resp.tile([128, NB], F32, tag="cosl_c")          # cos at label
            s30_c = resp.tile([128, NB], F32, tag="s30_c")            # 30/||e||

            # ---------------- phase 1: label path + weight prep ----------------
            with (
                tc.tile_pool(name="wraw", bufs=6) as wrawp,
                tc.tile_pool(name="prep", bufs=4) as prepp,
                tc.tile_pool(name="normp", bufs=2) as normp,
                tc.tile_pool(name="lab", bufs=3) as labp,
                tc.tile_pool(name="psum_ss", bufs=1, space="PSUM") as pssp,
            ):
                # label path: per 128-row batch tile, fused square/dot reduces
                for i in range(NB):
                    en = labp.tile([128, D], F32, tag="enat")
                    nc.sync.dma_start(en[:], emb_d.ap()[128 * i:128 * (i + 1), :])
                    wl = labp.tile([128, D], F32, tag="wlt")
                    nc.sync.dma_start(wl[:], wlab_d.ap()[128 * i:128 * (i + 1), :])
                    scr = labp.tile([128, D], F32, tag="scr")
                    nc.vector.scalar_tensor_tensor(
                        scr[:], en[:], 1.0, en[:], Alu.mult, Alu.mult,
                        accum_out=sse_c[:, i:i + 1])
                    scr2 = labp.tile([128, D], F32, tag="scr")
                    nc.vector.scalar_tensor_tensor(
                        scr2[:], wl[:], 1.0, wl[:], Alu.mult, Alu.mult,
                        accum_out=ssw_c[:, i:i + 1])
                    scr3 = labp.tile([128, D], F32, tag="scr")
                    nc.vector.scalar_tensor_tensor(
                        scr3[:], en[:], 1.0, wl[:], Alu.mult, Alu.mult,
                        accum_out=dot_c[:, i:i + 1])

                # batch-wide label math on [128, 16] tiles
                norm_e = resp.tile([128, NB], F32, tag="norm_e")
                nc.scalar.activation(norm_e[:], sse_c[:], Act.Sqrt)
                nc.vector.tensor_scalar(norm_e[:], norm_e[:], float(EPS), None, Alu.max)
                inve = resp.tile([128, NB], F32, tag="inve")
                nc.vector.reciprocal(inve[:], norm_e[:])
                norm_w = resp.tile([128, NB], F32, tag="norm_w")
                nc.scalar.activation(norm_w[:], ssw_c[:], Act.Sqrt)
                nc.vector.tensor_scalar(norm_w[:], norm_w[:], float(EPS), None, Alu.max)
                invwl = resp.tile([128, NB], F32, tag="invwl")
                nc.vector.reciprocal(invwl[:], norm_w[:])
                nc.vector.tensor_mul(cosl_c[:], dot_c[:], inve[:])
                nc.vector.tensor_mul(cosl_c[:], cosl_c[:], invwl[:])
                nc.vector.tensor_scalar(s30_c[:], inve[:], float(SCALE), None, Alu.mult)

                # embT load + bf16 cast
                for k in range(NK):
                    et = wrawp.tile([128, 2048], F32, tag="wtraw")
                    nc.sync.dma_start(et[:], embT_d.ap()[128 * k:128 * (k + 1), :])
                    nc.vector.tensor_copy(ebT_bf[:, k, :], et[:])

                # weight slabs: load, square, column-norm via ones-matmul,
                # normalize + cast to bf16
                for (soff, ssz) in SLABS:
                    ss_ps = pssp.tile([128, 2048], F32, tag="ss")
                    wts = []
                    for k in range(NK):
                        wt = wrawp.tile([128, 2048], F32, tag="wtraw")
                        nc.sync.dma_start(
                            wt[:, :ssz],
                            wT_d.ap()[128 * k:128 * (k + 1), soff:soff + ssz])
                        wts.append(wt)
                        wt2 = prepp.tile([128, 2048], BF16, tag="wt2")
                        nc.vector.tensor_mul(wt2[:, :ssz], wt[:, :ssz], wt[:, :ssz])
                        for n0 in range(0, ssz, 512):
                            nc.tensor.matmul(
                                ss_ps[:, n0:n0 + 512], ones_st[:],
                                wt2[:, n0:n0 + 512],
                                start=(k == 0), stop=(k == NK - 1))
                    nv = normp.tile([128, 2048], F32, tag="nv")
                    nc.scalar.activation(nv[:, :ssz], ss_ps[:, :ssz], Act.Sqrt)
                    nc.vector.tensor_scalar(nv[:, :ssz], nv[:, :ssz], float(EPS), None, Alu.max)
                    nc.vector.reciprocal(nv[:, :ssz], nv[:, :ssz])
                    for k in range(NK):
                        nc.vector.tensor_mul(
                            wtn_bf[:, k, soff:soff + ssz],
                            wts[k][:, :ssz], nv[:, :ssz])

            if stage == "prep":
                with tc.tile_pool(name="dbg", bufs=1) as dbgp:
                    dbg = dbgp.tile([1, 1], F32, tag="dbg")
                    nc.vector.tensor_copy(dbg[:], cosl_c[0:1, 0:1])
                    nc.sync.dma_start(out_d.ap()[:, :], dbg[:])
            if stage == "main":
                with tc.tile_pool(name="dbg", bufs=1) as dbgp:
                    dbg = dbgp.tile([1, 1], F32, tag="dbg")
                    nc.vector.tensor_copy(dbg[:], Pcols[0:1, 0:1])
                    nc.sync.dma_start(out_d.ap()[:, :], dbg[:])

            # ---------------- phase 2: main matmul + fused exp/row-sum ----------
            if stage != "prep":
              with tc.tile_pool(name="expo", bufs=4) as expop:
                for jji, (joff, jsz) in enumerate(JGROUPS):
                    for i in range(NB):
                        ps = pcosp.tile([128, 1024], F32, tag="cos")
                        for k in range(NK):
                            for h0 in range(0, jsz, 512):
                                nc.tensor.matmul(
                                    ps[:, h0:h0 + 512],
                                    ebT_bf[:, k, 128 * i:128 * (i + 1)],
                                    wtn_bf[:, k, joff + h0:joff + h0 + 512],
                                    start=(k == 0), stop=(k == NK - 1))
                        ex = expop.tile([128, 1024], BF16, tag="ex")
                        nc.scalar.activation(
                            ex[:, :jsz], ps[:, :jsz], Act.Exp,
                            bias=0.0, scale=s30_c[:, i:i + 1],
                            accum_out=Pcols[:, i * NJ + jji:i * NJ + jji + 1])

            # ---------------- phase 3: all-reduce + loss --------------------------
            if stage not in ("prep", "main"):
              with (
                tc.tile_pool(name="fin", bufs=1) as finp,
                tc.tile_pool(name="psum_fin", bufs=1, space="PSUM") as pfinp,
              ):
                P_loc = finp.tile([128, NB], F32, tag="P_loc")
                nc.vector.tensor_reduce(
                    P_loc[:], Pcols[:].rearrange("p (i j) -> p i j", j=NJ),
                    mybir.AxisListType.X, Alu.add)

                P_tot = finp.tile([128, NB], F32, tag="P_tot")
                if stage == "nocc":
                    nc.vector.tensor_copy(P_tot[:], P_loc[:])
                else:
                    cc_in = dramp.tile([128, NB], F32)
                    cc_out = dramp.tile([128, NB], F32, addr_space="Shared")
                    nc.gpsimd.dma_start(cc_in[:], P_loc[:])
                    nc.gpsimd.collective_compute(
                        "AllReduce", Alu.add,
                        replica_groups=[list(range(NCORES))],
                        ins=[cc_in[:].opt()], outs=[cc_out[:].opt()])
                    nc.gpsimd.dma_start(P_tot[:], cc_out[:])

                # margin correction: S = P_tot - npad - exp(30*cosl) + exp(30*cosl - 9)
                # where exp(30c-9) = exp(30c) * exp(-9), so corr = e1*(exp(-9)-1)
                e1 = finp.tile([128, NB], F32, tag="e1")
                nc.scalar.activation(e1[:], cosl_c[:], Act.Exp, bias=0.0, scale=float(SCALE))
                corr = finp.tile([128, NB], F32, tag="corr")
                nc.vector.tensor_scalar(
                    corr[:], e1[:], float(np.exp(-MARGIN * SCALE) - 1.0), None, Alu.mult)
                S = finp.tile([128, NB], F32, tag="S")
                nc.vector.tensor_scalar(S[:], P_tot[:], float(-PAD * NCORES), None, Alu.add)
                nc.vector.tensor_add(S[:], S[:], corr[:])
                lnS = finp.tile([128, NB], F32, tag="lnS")
                nc.scalar.activation(lnS[:], S[:], Act.Ln)
                tgt = finp.tile([128, NB], F32, tag="tgt")
                nc.vector.tensor_scalar(
                    tgt[:], cosl_c[:], float(SCALE), float(-MARGIN * SCALE),
                    Alu.mult, Alu.add)
                nll = finp.tile([128, NB], F32, tag="nll")
                nc.vector.tensor_sub(nll[:], lnS[:], tgt[:])
                nrow = finp.tile([128, 1], F32, tag="nrow")
                nc.vector.tensor_reduce(nrow[:], nll[:], mybir.AxisListType.X, Alu.add)

                ps11 = pfinp.tile([1, 1], F32, tag="ps11")
                nc.tensor.matmul(ps11[:], ones_col[:], nrow[:], start=True, stop=True)
                loss_sb = finp.tile([1, 1], F32, tag="loss_sb")
                nc.scalar.mul(loss_sb[:], ps11[:], 1.0 / B)
                nc.sync.dma_start(out_d.ap()[:, :], loss_sb[:])

    nc.compile()
    nc.m = get_hw_module(nc.m)
    return nc


_NC_CACHE = None


def _get_nc():
    global _NC_CACHE
    if _NC_CACHE is None:
        import os
        _NC_CACHE = build(stage=os.environ.get("KERNEL_STAGE", "full"))
    return _NC_CACHE


def make_in_maps(embeddings, labels, weight):
    embeddings = np.ascontiguousarray(np.asarray(embeddings, dtype=np.float32))
    weight = np.ascontiguousarray(np.asarray(weight, dtype=np.float32))
    labels_i = np.asarray(labels).astype(np.int64)

    embT = np.ascontiguousarray(embeddings.T)
    wlab = np.ascontiguousarray(weight[labels_i])

    in_maps = []
    for c in range(NCORES):
        shard = weight[c * CS:(c + 1) * CS]               # [6250, 512]
        wT = np.zeros((D, CSP), dtype=np.float32)
        wT[:, :CS] = shard.T
        in_maps.append({"embT": embT, "emb": embeddings, "wlab": wlab, "wT": wT})
    return in_maps


def kernel(embeddings, labels, weight, _trace=False, _trace_kwargs=None):
    in_maps = make_in_maps(embeddings, labels, weight)
    nc = _get_nc()
    res = bass_utils.run_bass_kernel_spmd(
        nc, in_maps, core_ids=list(range(NCORES)),
        trace=_trace, **(_trace_kwargs or {}))
    out = np.asarray(res.results[0]["out"], dtype=np.float32).reshape(())
    if _trace:
        kernel.last_result = res
    return out


# revision 14
# speedup vs baseline: 1.1034x; 1.1034x over previous
"""ArcFace loss kernel for 8 TRN2 NeuronCores.

Strategy: tensor-parallel over classes (C=50000 -> 6250/core, padded to
6656 = 13*512).  Each core computes cos(emb, w_shard) with a bf16 matmul
and a fused exp+row-sum epilogue on the scalar engine (per-row 1/||e||
folded into the activation scale).  Row/label norms and the label logit
come from Gram-diagonal matmuls on the transposed operands.  A single
8 KB AllReduce combines the per-core sum-exp vectors; the margin-corrected
log-softmax NLL mean is computed redundantly on every core.
"""

import numpy as np

from concourse import bacc, bass, mybir, tile
from concourse import bass_utils
from concourse.bass_interp import get_hw_module
from concourse.masks import make_identity

B, D, C = 2048, 512, 50000
NCORES = 8
CS = C // NCORES            # 6250 classes per core
CSP = 6656                  # padded to 13*512
PAD = CSP - CS              # 406 zero-padded classes per core
MARGIN = 0.3
SCALE = 30.0
EPS = 1e-12

F32 = mybir.dt.float32
BF16 = mybir.dt.bfloat16
Act = mybir.ActivationFunctionType
Alu = mybir.AluOpType

NB = B // 128               # 16 batch tiles
NK = D // 128               # 4 contraction tiles
# main-loop column groups (pairs of 512 -> 1024-wide psum tiles)
JGROUPS = [(o, min(1024, CSP - o)) for o in range(0, CSP, 1024)]  # 6x1024 + 1x512
NJ = len(JGROUPS)           # 7
# weight-prep slabs of 2048 (ss/normalize processed in 1024-halves)
SLABS = [(o, min(2048, CSP - o)) for o in range(0, CSP, 2048)]    # 3x2048 + 1x512


def build(stage="full"):
    nc = bacc.Bacc("TRN2", debug=False, num_devices=NCORES)

    embT_d = nc.dram_tensor("embT", [D, B], F32, kind="ExternalInput")
    wlabT_d = nc.dram_tensor("wlabT", [D, B], F32, kind="ExternalInput")
    wT_d = nc.dram_tensor("wT", [D, CSP], F32, kind="ExternalInput")
    out_d = nc.dram_tensor("out", [1, 1], F32, kind="ExternalOutput")

    with tile.TileContext(nc) as tc:
        with (
            tc.tile_pool(name="const", bufs=1) as constp,
            tc.tile_pool(name="res", bufs=1) as resp,
            tc.tile_pool(name="psum_cos", bufs=2, space="PSUM") as pcosp,
            tc.tile_pool(name="psum_aux", bufs=1, space="PSUM") as pauxp,
            tc.tile_pool(name="dram", bufs=1, space="DRAM") as dramp,
            tc.tile_pool(name="wraw", bufs=6) as wrawp,
            tc.tile_pool(name="prep", bufs=4) as prepp,
            tc.tile_pool(name="normp", bufs=2) as normp,
        ):
            ones_st = constp.tile([128, 128], BF16, tag="ones_st")
            nc.vector.memset(ones_st[:], 1.0)
            ones_col = constp.tile([128, 1], F32, tag="ones_col")
            nc.vector.memset(ones_col[:], 1.0)
            # identity mask for Gram-diagonal extraction
            ident = constp.tile([128, 128], F32, tag="ident")
            make_identity(nc, ident[:])

            # resident tensors
            ebT_bf = resp.tile([128, NK, B], BF16, tag="ebT_bf")      # 16 KB/part
            wlT_bf = resp.tile([128, NK, B], BF16, tag="wlT_bf")      # 16 KB/part
            wtn_bf = resp.tile([128, NK, CSP], BF16, tag="wtn_bf")    # 53 KB/part
            Pcols = resp.tile([128, NB * NJ], F32, tag="Pcols")       # exp-sum accums
            sse_c = resp.tile([128, NB], F32, tag="sse_c")            # ||e||^2
            ssw_c = resp.tile([128, NB], F32, tag="ssw_c")            # ||w_lab||^2
            dot_c = resp.tile([128, NB], F32, tag="dot_c")            # e . w_lab
            cosl_c = resp.tile([128, NB], F32, tag="cosl_c")          # cos at label
            s30_c = resp.tile([128, NB], F32, tag="s30_c")            # 30/||e||
            inve_c = resp.tile([128, NB], F32, tag="inve_c")          # 1/||e||

            # warm-up collective: tiny AR so ncfw/SPAD is staged before the real one
            warm_in = dramp.tile([128, 1], F32, name="warm_in")
            warm_out = dramp.tile([128, 1], F32, name="warm_out", addr_space="Shared")
            nc.gpsimd.dma_start(warm_in[:], ones_col[:])
            nc.gpsimd.collective_compute(
                "AllReduce", Alu.add, replica_groups=[list(range(NCORES))],
                ins=[warm_in[:].opt()], outs=[warm_out[:].opt()])

            # ---- embT load + cast (gates both main matmul and exp scale) ----
            for k in range(NK):
                et = wrawp.tile([128, 2048], F32, tag="wtraw")
                nc.sync.dma_start(et[:], embT_d.ap()[128 * k:128 * (k + 1), :])
                nc.vector.tensor_copy(ebT_bf[:, k, :], et[:])

            # ---- per-batch-tile row norms via Gram diagonal: gates exp ----
            for i in range(NB):
                gps = pauxp.tile([128, 128], F32, tag="gram", bufs=1)
                for k in range(NK):
                    nc.tensor.matmul(
                        gps[:], ebT_bf[:, k, 128 * i:128 * (i + 1)],
                        ebT_bf[:, k, 128 * i:128 * (i + 1)],
                        start=(k == 0), stop=(k == NK - 1))
                gsc = prepp.tile([128, 128], F32, tag="gsc")
                nc.vector.scalar_tensor_tensor(
                    gsc[:], gps[:], 1.0, ident[:], Alu.mult, Alu.mult,
                    accum_out=sse_c[:, i:i + 1])
                nc.scalar.activation(s30_c[:, i:i + 1], sse_c[:, i:i + 1], Act.Sqrt)
                nc.vector.tensor_scalar(
                    s30_c[:, i:i + 1], s30_c[:, i:i + 1], float(EPS), None, Alu.max)
                nc.vector.reciprocal(inve_c[:, i:i + 1], s30_c[:, i:i + 1])
                nc.vector.tensor_scalar(
                    s30_c[:, i:i + 1], inve_c[:, i:i + 1], float(SCALE), None, Alu.mult)

            # ---- weight slabs: load, square, column-norms, normalize+cast ----
            for (soff, ssz) in SLABS:
                wts = []
                wt2s = []
                for k in range(NK):
                    wt = wrawp.tile([128, 2048], F32, tag="wtraw")
                    nc.sync.dma_start(
                        wt[:, :ssz],
                        wT_d.ap()[128 * k:128 * (k + 1), soff:soff + ssz])
                    wts.append(wt)
                    wt2 = prepp.tile([128, 2048], BF16, tag="wt2")
                    nc.vector.tensor_mul(wt2[:, :ssz], wt[:, :ssz], wt[:, :ssz])
                    wt2s.append(wt2)
                for h0 in range(0, ssz, 1024):
                    hsz = min(1024, ssz - h0)
                    ss_ps = pauxp.tile([128, 1024], F32, tag="ss", bufs=1)
                    for k in range(NK):
                        for n0 in range(0, hsz, 512):
                            nc.tensor.matmul(
                                ss_ps[:, n0:n0 + 512], ones_st[:],
                                wt2s[k][:, h0 + n0:h0 + n0 + 512],
                                start=(k == 0), stop=(k == NK - 1))
                    nv = normp.tile([128, 1024], F32, tag="nv")
                    nc.scalar.activation(nv[:, :hsz], ss_ps[:, :hsz], Act.Sqrt)
                    nc.vector.tensor_scalar(nv[:, :hsz], nv[:, :hsz], float(EPS), None, Alu.max)
                    nc.vector.reciprocal(nv[:, :hsz], nv[:, :hsz])
                    for k in range(NK):
                        nc.vector.tensor_mul(
                            wtn_bf[:, k, soff + h0:soff + h0 + hsz],
                            wts[k][:, h0:h0 + hsz], nv[:, :hsz])

            # ---- main loop: cos matmul + fused exp/row-sum ----
            if stage != "prep":
                with tc.tile_pool(name="expo", bufs=4) as expop:
                    for jji, (joff, jsz) in enumerate(JGROUPS):
                        for i in range(NB):
                            ps = pcosp.tile([128, 1024], F32, tag="cos")
                            for k in range(NK):
                                for h0 in range(0, jsz, 512):
                                    nc.tensor.matmul(
                                        ps[:, h0:h0 + 512],
                                        ebT_bf[:, k, 128 * i:128 * (i + 1)],
                                        wtn_bf[:, k, joff + h0:joff + h0 + 512],
                                        start=(k == 0), stop=(k == NK - 1))
                            ex = expop.tile([128, 1024], BF16, tag="ex")
                            nc.scalar.activation(
                                ex[:, :jsz], ps[:, :jsz], Act.Exp,
                                bias=0.0, scale=s30_c[:, i:i + 1],
                                accum_out=Pcols[:, i * NJ + jji:i * NJ + jji + 1])

            # ---- late label path: wlabT norms + label dot (Gram diagonals) ----
            for k in range(NK):
                wlt = wrawp.tile([128, 2048], F32, tag="wtraw")
                nc.sync.dma_start(wlt[:], wlabT_d.ap()[128 * k:128 * (k + 1), :])
                nc.vector.tensor_copy(wlT_bf[:, k, :], wlt[:])
            for i in range(NB):
                gps2 = pauxp.tile([128, 256], F32, tag="gram", bufs=1)
                for k in range(NK):
                    nc.tensor.matmul(
                        gps2[:, 0:128], wlT_bf[:, k, 128 * i:128 * (i + 1)],
                        wlT_bf[:, k, 128 * i:128 * (i + 1)],
                        start=(k == 0), stop=(k == NK - 1))
                for k in range(NK):
                    nc.tensor.matmul(
                        gps2[:, 128:256], ebT_bf[:, k, 128 * i:128 * (i + 1)],
                        wlT_bf[:, k, 128 * i:128 * (i + 1)],
                        start=(k == 0), stop=(k == NK - 1))
                gsc2 = prepp.tile([128, 128], F32, tag="gsc")
                nc.vector.scalar_tensor_tensor(
                    gsc2[:], gps2[:, 0:128], 1.0, ident[:], Alu.mult, Alu.mult,
                    accum_out=ssw_c[:, i:i + 1])
                gsc3 = prepp.tile([128, 128], F32, tag="gsc")
                nc.vector.scalar_tensor_tensor(
                    gsc3[:], gps2[:, 128:256], 1.0, ident[:], Alu.mult, Alu.mult,
                    accum_out=dot_c[:, i:i + 1])

            # batched label math (needed only for the epilogue)
            norm_w = resp.tile([128, NB], F32, tag="norm_w")
            nc.scalar.activation(norm_w[:], ssw_c[:], Act.Sqrt)
            nc.vector.tensor_scalar(norm_w[:], norm_w[:], float(EPS), None, Alu.max)
            invwl = resp.tile([128, NB], F32, tag="invwl")
            nc.vector.reciprocal(invwl[:], norm_w[:])
            nc.vector.tensor_mul(cosl_c[:], dot_c[:], inve_c[:])
            nc.vector.tensor_mul(cosl_c[:], cosl_c[:], invwl[:])

            # ---- all-reduce + loss ----
            if stage == "full":
                with (
                    tc.tile_pool(name="fin", bufs=1) as finp,
                    tc.tile_pool(name="psum_fin", bufs=1, space="PSUM") as pfinp,
                ):
                    P_loc = finp.tile([128, NB], F32, tag="P_loc")
                    nc.vector.tensor_reduce(
                        P_loc[:], Pcols[:].rearrange("p (i j) -> p i j", j=NJ),
                        mybir.AxisListType.X, Alu.add)

                    cc_in = dramp.tile([128, NB], F32, name="cc_in")
                    cc_out = dramp.tile([128, NB], F32, name="cc_out", addr_space="Shared")
                    nc.gpsimd.dma_start(cc_in[:], P_loc[:])
                    nc.gpsimd.collective_compute(
                        "AllReduce", Alu.add,
                        replica_groups=[list(range(NCORES))],
                        ins=[cc_in[:].opt()], outs=[cc_out[:].opt()])
                    P_tot = finp.tile([128, NB], F32, tag="P_tot")
                    nc.gpsimd.dma_start(P_tot[:], cc_out[:])

                    # margin: S = P_tot - npad - exp(30*cosl) + exp(30*cosl - 9)
                    e1 = finp.tile([128, NB], F32, tag="e1")
                    nc.scalar.activation(e1[:], cosl_c[:], Act.Exp,
                                         bias=0.0, scale=float(SCALE))
                    corr = finp.tile([128, NB], F32, tag="corr")
                    nc.vector.tensor_scalar(
                        corr[:], e1[:], float(np.exp(-MARGIN * SCALE) - 1.0),
                        None, Alu.mult)
                    S = finp.tile([128, NB], F32, tag="S")
                    nc.vector.tensor_scalar(
                        S[:], P_tot[:], float(-PAD * NCORES), None, Alu.add)
                    nc.vector.tensor_add(S[:], S[:], corr[:])
                    lnS = finp.tile([128, NB], F32, tag="lnS")
                    nc.scalar.activation(lnS[:], S[:], Act.Ln)
                    tgt = finp.tile([128, NB], F32, tag="tgt")
                    nc.vector.tensor_scalar(
                        tgt[:], cosl_c[:], float(SCALE), float(-MARGIN * SCALE),
                        Alu.mult, Alu.add)
                    nll = finp.tile([128, NB], F32, tag="nll")
                    nc.vector.tensor_sub(nll[:], lnS[:], tgt[:])
                    nrow = finp.tile([128, 1], F32, tag="nrow")
                    nc.vector.tensor_reduce(
                        nrow[:], nll[:], mybir.AxisListType.X, Alu.add)

                    ps11 = pfinp.tile([1, 1], F32, tag="ps11")
                    nc.tensor.matmul(ps11[:], ones_col[:], nrow[:],
                                     start=True, stop=True)
                    loss_sb = finp.tile([1, 1], F32, tag="loss_sb")
                    nc.scalar.mul(loss_sb[:], ps11[:], 1.0 / B)
                    nc.sync.dma_start(out_d.ap()[:, :], loss_sb[:])

    nc.compile()
    nc.m = get_hw_module(nc.m)
    return nc


_NC_CACHE = None


def _get_nc():
    global _NC_CACHE
    if _NC_CACHE is None:
        import os
        _NC_CACHE = build(stage=os.environ.get("KERNEL_STAGE", "full"))
    return _NC_CACHE


def make_in_maps(embeddings, labels, weight):
    embeddings = np.ascontiguousarray(np.asarray(embeddings, dtype=np.float32))
    weight = np.ascontiguousarray(np.asarray(weight, dtype=np.float32))
    labels_i = np.asarray(labels).astype(np.int64)

    embT = np.ascontiguousarray(embeddings.T)
    wlabT = np.ascontiguousarray(weight[labels_i].T)

    in_maps = []
    for c in range(NCORES):
        shard = weight[c * CS:(c + 1) * CS]               # [6250, 512]
        wT = np.zeros((D, CSP), dtype=np.float32)
        wT[:, :CS] = shard.T
        in_maps.append({"embT": embT, "wlabT": wlabT, "wT": wT})
    return in_maps


def kernel(embeddings, labels, weight, _trace=False, _trace_kwargs=None):
    in_maps = make_in_maps(embeddings, labels, weight)
    nc = _get_nc()
    res = bass_utils.run_bass_kernel_spmd(
        nc, in_maps, core_ids=list(range(NCORES)),
        trace=_trace, **(_trace_kwargs or {}))
    out = np.asarray(res.results[0]["out"], dtype=np.float32).reshape(())
    if _trace:
        kernel.last_result = res
    return out
